# revision 28
# baseline (speedup 1.0000x reference)
"""BiMamba block kernel for 8 Trainium2 NeuronCores.

Sharding: core = 2*sample + direction (4 samples x 2 scan directions).
Each core runs the full mamba for its (sample, direction).

The selective scan dominates on DVE (16 states x 4096 cols at ~2.1
ns/col is irreducible), so the kernel keeps the DVE queue dense and
hides everything else under it:

- All activations use only the natural_log_exp ACT table set: silu is
  computed as v*sigmoid(v) with sigmoid(v) = exp(-softplus(-v))
  (3 exp/ln ops + one DVE scalar_tensor_tensor), so no table reload
  ever interrupts the exp stream of the scan.
- Scan: 4 segments (1024/1024/1536/512 cols); the first scan starts as
  soon as front chunks 0-1 are projected (~20 us).  Remaining front
  chunks are interleaved between scan states.  B/C rows arrive via
  DRAM partition-broadcast DMAs issued 5 states ahead.
- Exchange: after segments 1/2/3 the out-projection rows are
  AllGathered with the pair core (rank order = [dir0; dir1], matching
  the reference's un-unflipped y2 concat), giving every core the full
  128-channel conv input; the 3x3 conv runs locally, no collective
  after it.  A dummy 8-core AllReduce early in the kernel absorbs the
  core launch skew so the tail stats AllReduce doesn't pay it.
- BatchNorm stats are split: chunks 0-5 AllReduce overlaps the last
  conv chunks; only the tiny chunk 6-7 reduce sits on the tail.
  invstd via ln/exp (no sqrt table load).
"""
import os
import sys

for _p in ("/opt/trn_rl_repo", "/root/.axon_site/_ro/trn_rl_repo"):
    if os.path.isdir(_p):
        if _p not in sys.path:
            sys.path.insert(0, _p)
        break

import ml_dtypes
import numpy as np

# The agent image's antenv lacks axon_hooks; inject it so trace=True can
# capture NTFF profiles (used by test.py for HW timing, not for grading).
try:
    import antenv.axon_hooks  # noqa: F401
except ImportError:
    try:
        import types as _types

        from trn_agent_boot.trn_boot import _ntff_profile_via_ctypes

        _hook = _ntff_profile_via_ctypes("/opt/axon/libaxon_pjrt.so")
        _m = _types.ModuleType("antenv.axon_hooks")
        _m.get_axon_ntff_profile_hook = lambda: _hook
        _m.set_axon_ntff_profile_hook = lambda h: None
        sys.modules["antenv.axon_hooks"] = _m
    except Exception:
        pass

import concourse.bass as bass
import concourse.mybir as mybir
from concourse import bacc
from concourse import bass_utils
from concourse.masks import make_identity
from concourse.tile import TileContext

F32 = mybir.dt.float32
BF16 = mybir.dt.bfloat16
AF = mybir.ActivationFunctionType
OP = mybir.AluOpType

B, C, H, W = 4, 64, 64, 64
L = H * W          # 4096
DI = 128           # d_inner
DS = 16            # d_state
DTR = 4            # dt_rank
DCONV = 4
NCORE = 8
CH = 512           # matmul free-dim chunk
NCH = L // CH      # 8
RPC = CH // W      # output rows per chunk (8)

SEGS = ((0, 1024), (1024, 2048), (2048, 3584), (3584, 4096))
SEG_CHUNKS = ((0, 1), (2, 3), (4, 5, 6), (7,))
NSEG = len(SEGS)
# exchange waves: (after_seg, chunks)
WAVES = ((1, (0, 1, 2, 3)), (2, (4, 5, 6)), (3, (7,)))

BH_COLS = 9 * C + C + 128 + 32   # c3w | owT | bigT | bcwT
BF_COLS = 736


def _build():
    nc = bacc.Bacc(target_bir_lowering=False, debug=False, num_devices=NCORE)

    def din(name, shape, dtype=F32):
        return nc.dram_tensor(name, shape, dtype, kind="ExternalInput")

    F32R = mybir.dt.float32r
    x_loc = din("x_loc", [C, L], F32R)
    blob_f = din("blob_f", [128, BF_COLS], F32R)
    blob_h = din("blob_h", [128, BH_COLS], BF16)

    out_d = nc.dram_tensor("out", [C, L], F32, kind="ExternalOutput")

    with TileContext(nc) as tc:
        with tc.tile_pool(name="pers", bufs=1) as pers:
            # ---- params arrive as two packed blobs ----
            p_bf = pers.tile([128, BF_COLS], F32R)
            p_bh = pers.tile([128, BH_COLS], BF16)
            nc.sync.dma_start(p_bf[:], blob_f[:])
            nc.sync.dma_start(p_bh[:], blob_h[:])
            p_wk = [p_bf[:, 128 * k:128 * (k + 1)] for k in range(DCONV)]
            p_zwT = p_bf[:, 512:640]
            p_rwT = p_bf[:, 640:704]
            p_c1b = p_bf[:, 704:705].bitcast(F32)
            p_dtb = p_bf[:, 705:706].bitcast(F32)
            p_A = p_bf[:, 706:722].bitcast(F32)
            p_D = p_bf[:, 722:723].bitcast(F32)
            p_c3b = p_bf[:C, 723:724].bitcast(F32)
            p_rb = p_bf[:C, 724:725].bitcast(F32)
            p_bng = p_bf[:C, 725:726].bitcast(F32)
            p_bnb = p_bf[:C, 726:727].bitcast(F32)
            p_nc1b = p_bf[:, 728:729].bitcast(F32)   # -conv1 bias
            p_c3w = p_bh[:, 0:9 * C]
            p_owT = p_bh[:, 9 * C:9 * C + C]
            p_bigT = p_bh[:, 9 * C + C:9 * C + C + 128]
            p_bcwT = p_bh[:, 9 * C + C + 128:9 * C + C + 160]

            ident = pers.tile([128, 128], F32)
            make_identity(nc, ident[:])
            ident_g = pers.tile([128, 128], BF16)
            nc.vector.tensor_copy(ident_g[:], ident[:])

            # DRAM staging for B/C rows (DMA partition-broadcast needs a
            # DRAM source)
            bc_dram = nc.dram_tensor("bc_stage", [2 * DS, L], BF16)

            x_pad = pers.tile([64, 3 + L], F32R)
            nc.gpsimd.memset(x_pad[:, 0:3].bitcast(F32), 0.0)
            # split load so chunk-0/1 matmuls start early (chunk 1's last
            # tap reads through x col 1024)
            nc.sync.dma_start(x_pad[:, 3:3 + 1040], x_loc[:, 0:1040])
            nc.sync.dma_start(x_pad[:, 3 + 1040:3 + L], x_loc[:, 1040:L])

            with tc.tile_pool(name="smid", bufs=1) as smid, \
                 tc.tile_pool(name="ps", bufs=4, space="PSUM") as psp, \
                 tc.tile_pool(name="psy", bufs=4, space="PSUM") as psy, \
                 tc.tile_pool(name="sl_e", bufs=2) as plex, \
                 tc.tile_pool(name="sl_a", bufs=3) as pla, \
                 tc.tile_pool(name="sl_b", bufs=5) as plb, \
                 tc.tile_pool(name="sl_x", bufs=3) as plx, \
                 tc.tile_pool(name="sl_h", bufs=3) as plh, \
                 tc.tile_pool(name="sl_c", bufs=5) as plc, \
                 tc.tile_pool(name="sl_g", bufs=3) as plg, \
                 tc.tile_pool(name="sl_f", bufs=2) as plf, \
                 tc.tile_pool(name="dram", bufs=1, space="DRAM") as dr:
                z_sil = smid.tile([DI, L], BF16)
                dtv = smid.tile([DI, L], BF16)
                dtxc = smid.tile([DI, L], BF16)
                xcd = smid.tile([DI, L], BF16)
                xc = smid.tile([DI, L], BF16)
                carry = smid.tile([DI, DS], F32)

                ympad = smid.tile([128, H + 2, W + 2], BF16)
                nc.gpsimd.memset(ympad[:], 0.0)
                res_sb = smid.tile([C, L], BF16)
                conv_sb = smid.tile([C, L], BF16)
                stats_m = smid.tile([C, NCH], F32)
                stats_v = smid.tile([C, NCH], F32)
                PAIRS = [[0, 1], [2, 3], [4, 5], [6, 7]]
                G8 = [[0, 1, 2, 3, 4, 5, 6, 7]]

                cc_ins, cc_outs = [], []
                for wi, (_, cvs) in enumerate(WAVES):
                    cc_ins.append(dr.tile([C, len(cvs) * CH], BF16,
                                          name=f"cci{wi}"))
                    cc_outs.append(dr.tile([128, len(cvs) * CH], BF16,
                                           name=f"cco{wi}"))
                st_in_a = dr.tile([C, 2], F32, name="st_in_a")
                st_in_b = dr.tile([C, 2], F32, name="st_in_b")
                sync_in = dr.tile([C, 2], F32, name="sync_in")
                st_out_a = nc.dram_tensor("st_out_a", [C, 2], F32,
                                          addr_space="Shared")
                st_out_b = nc.dram_tensor("st_out_b", [C, 2], F32,
                                          addr_space="Shared")
                sync_out = nc.dram_tensor("sync_out", [C, 2], F32,
                                          addr_space="Shared")

                def sigmoid_mul(dst_sl, ps_t, bias, nbias):
                    """dst = (v+b)*sigmoid(v+b) with sigmoid computed as
                    exp(-ln(1+exp(-(v+b)))) -- exp/ln only, one table set."""
                    t = plf.tile([DI, CH], BF16, tag="sg")
                    if nbias is not None:
                        nc.scalar.activation(t[:], ps_t[:DI], AF.Exp,
                                             scale=-1.0, bias=nbias[:, 0:1])
                    else:
                        nc.scalar.activation(t[:], ps_t[:DI], AF.Exp,
                                             scale=-1.0)
                    nc.scalar.activation(t[:], t[:], AF.Ln, bias=1.0)
                    nc.scalar.activation(t[:], t[:], AF.Exp, scale=-1.0)
                    nc.vector.scalar_tensor_tensor(
                        dst_sl, ps_t[:DI],
                        bias[:, 0:1] if bias is not None else 0.0,
                        t[:], op0=OP.add, op1=OP.mult)

                def front_chunk(c, with_ln):
                    """Full front for chunk c: fused in-proj + causal conv,
                    gate, silu, dt/B/C projections."""
                    sl = slice(c * CH, (c + 1) * CH)
                    ps = psp.tile([128, CH], F32, tag="ps")
                    for k in range(DCONV):
                        nc.tensor.matmul(ps[:DI], p_wk[k][:C],
                                         x_pad[:, c * CH + k:c * CH + k + CH],
                                         start=(k == 0), stop=(k == DCONV - 1))
                    ps2 = psp.tile([128, CH], F32, tag="ps")
                    nc.tensor.matmul(ps2[:DI], p_zwT[:C],
                                     x_pad[:, 3 + c * CH:3 + (c + 1) * CH],
                                     start=True, stop=True)
                    sigmoid_mul(xc[:, sl], ps, p_c1b, p_nc1b)
                    sigmoid_mul(z_sil[:, sl], ps2, None, None)
                    ps3 = psp.tile([128, CH], F32, tag="ps")
                    nc.tensor.matmul(ps3[:DI], p_bigT[:], xc[:, sl],
                                     start=True, stop=True)
                    nc.scalar.activation(dtv[:, sl], ps3[:DI], AF.Exp,
                                         bias=p_dtb[:, 0:1])
                    ps4 = psp.tile([128, CH], F32, tag="ps")
                    nc.tensor.matmul(ps4[:2 * DS], p_bcwT[:], xc[:, sl],
                                     start=True, stop=True)
                    bch = plb.tile([2 * DS, CH], BF16, tag="bch")
                    nc.scalar.copy(bch[:], ps4[:2 * DS])
                    nc.sync.dma_start(bc_dram[:, sl], bch[:])
                    if with_ln:
                        nc.scalar.activation(dtv[:, sl], dtv[:, sl], AF.Ln,
                                             bias=1.0)
                        nc.vector.tensor_mul(dtxc[:, sl], dtv[:, sl],
                                             xc[:, sl])
                        nc.scalar.activation(xcd[:, sl], xc[:, sl],
                                             AF.Copy, scale=p_D[:, 0:1])

                def finish_front(cs):
                    hsl = slice(cs[0] * CH, (cs[-1] + 1) * CH)
                    nc.scalar.activation(dtv[:, hsl], dtv[:, hsl], AF.Ln,
                                         bias=1.0)
                    for c in cs:
                        sl = slice(c * CH, (c + 1) * CH)
                        nc.vector.tensor_mul(dtxc[:, sl], dtv[:, sl],
                                             xc[:, sl])
                        nc.scalar.activation(xcd[:, sl], xc[:, sl],
                                             AF.Copy, scale=p_D[:, 0:1])

                def wave(wi):
                    """Out-projection + pair AllGather + ympad write +
                    residual for the wave's chunks.  AllGather output is
                    rank-ordered, so both cores get [dir0; dir1]."""
                    cvs = WAVES[wi][1]
                    stage = plex.tile([C, len(cvs) * CH], BF16,
                                      tag="stage", name=f"stage{wi}")
                    for j, cix in enumerate(cvs):
                        sl = slice(cix * CH, (cix + 1) * CH)
                        ssl = slice(j * CH, (j + 1) * CH)
                        yg = plf.tile([DI, CH], BF16, tag="yg")
                        nc.vector.tensor_mul(yg[:], y_ps[cix][:DI],
                                             z_sil[:, sl])
                        po = psp.tile([128, CH], F32, tag="ps",
                                      name=f"po{cix}")
                        nc.tensor.matmul(po[:C], p_owT[:], yg[:],
                                         start=True, stop=True)
                        nc.scalar.copy(stage[:, ssl], po[:C])
                        psr = psp.tile([128, CH], F32, tag="ps",
                                       name=f"rs{cix}")
                        nc.tensor.matmul(psr[:C], p_rwT[:C],
                                         x_pad[:, 3 + cix * CH:
                                               3 + (cix + 1) * CH],
                                         start=True, stop=True)
                        nc.scalar.activation(res_sb[:, sl], psr[:C],
                                             AF.Identity, bias=p_rb[:, 0:1])
                    nc.sync.dma_start(cc_ins[wi][:], stage[:])
                    nc.gpsimd.collective_compute(
                        "AllGather", OP.bypass, replica_groups=PAIRS,
                        ins=[cc_ins[wi][:].opt()], outs=[cc_outs[wi][:].opt()])
                    r0 = cvs[0] * RPC
                    nrows = len(cvs) * RPC
                    nc.sync.dma_start(
                        ympad[:, 1 + r0:1 + r0 + nrows, 1:1 + W],
                        cc_outs[wi][:].rearrange("p (r w) -> p r w", w=W))

                def conv3_chunk(c):
                    ps = psp.tile([128, CH], F32, tag="ps", name=f"cv{c}")
                    ps3 = ps[:C].rearrange("p (r w) -> p r w", w=W)
                    r0 = c * RPC
                    n = 0
                    for ky in range(3):
                        for kx in range(3):
                            nc.tensor.matmul(
                                ps3[:],
                                p_c3w[:, (ky * 3 + kx) * C:
                                      (ky * 3 + kx + 1) * C],
                                ympad[:, r0 + ky:r0 + ky + RPC, kx:kx + W],
                                start=(n == 0), stop=(n == 8))
                            n += 1
                    sl = slice(c * CH, (c + 1) * CH)
                    flat = ps3.rearrange("p r w -> p (r w)")
                    nc.scalar.activation(conv_sb[:, sl], flat,
                                         AF.Identity, bias=p_c3b[:, 0:1],
                                         accum_out=stats_m[:, c:c + 1])
                    sq = plf.tile([C, CH], BF16, tag="sq")
                    nc.scalar.activation(sq[:], conv_sb[:, sl],
                                         AF.Square,
                                         accum_out=stats_v[:, c:c + 1])

                stats_a = smid.tile([C, 2], F32)

                def stats_early():
                    nc.vector.tensor_reduce(stats_a[:, 0:1],
                                            stats_m[:, 0:6],
                                            axis=mybir.AxisListType.X,
                                            op=OP.add)
                    nc.vector.tensor_reduce(stats_a[:, 1:2],
                                            stats_v[:, 0:6],
                                            axis=mybir.AxisListType.X,
                                            op=OP.add)
                    nc.sync.dma_start(st_in_a[:], stats_a[:])
                    nc.gpsimd.collective_compute(
                        "AllReduce", OP.add, replica_groups=G8,
                        ins=[st_in_a[:].opt()], outs=[st_out_a[:].opt()])

                y_ps = {}
                bc_q = {}
                PREF = 5

                def bc_issue(q, s):
                    t0, t1 = SEGS[q]
                    SEG = t1 - t0
                    qsl = slice(t0, t1)
                    bbc = plb.tile([DI, SEG], BF16, tag="bbc",
                                   name=f"bbc{q}_{s}")
                    nc.sync.dma_start(
                        bbc[:],
                        bc_dram[s:s + 1, qsl].to_broadcast((DI, SEG)))
                    cbc = plc.tile([DI, SEG], BF16, tag="cbc",
                                   name=f"cbc{q}_{s}")
                    nc.sync.dma_start(
                        cbc[:],
                        bc_dram[DS + s:DS + s + 1, qsl].to_broadcast(
                            (DI, SEG)))
                    bc_q[(q, s)] = (bbc, cbc)

                def seg_prefetch(q):
                    for s in range(PREF):
                        bc_issue(q, s)

                def seg_scan(q, work):
                    t0, t1 = SEGS[q]
                    SEG = t1 - t0
                    qsl = slice(t0, t1)
                    for cix in SEG_CHUNKS[q]:
                        yp = psy.tile([128, CH], F32, tag="yps",
                                      name=f"y{cix}")
                        nc.tensor.matmul(yp[:DI], ident_g[:],
                                         xcd[:, cix * CH:(cix + 1) * CH],
                                         start=True, stop=False)
                        y_ps[cix] = yp
                    for s in range(DS):
                        da = pla.tile([DI, SEG], BF16, tag="da")
                        nc.scalar.activation(da[:], dtv[:, qsl], AF.Exp,
                                             scale=p_A[:, s:s + 1])
                        bbc, cbc = bc_q.pop((q, s))
                        dbx = plx.tile([DI, SEG], BF16, tag="dbx")
                        nc.vector.tensor_mul(dbx[:], dtxc[:, qsl], bbc[:])
                        h = plh.tile([DI, SEG], BF16, tag="h")
                        init = 0.0 if q == 0 else carry[:, s:s + 1]
                        nc.vector.tensor_tensor_scan(h[:], da[:], dbx[:],
                                                     init, op0=OP.mult,
                                                     op1=OP.add)
                        if q < NSEG - 1:
                            # on DVE so the in-order ACT queue of exps is
                            # never blocked behind a scan result
                            nc.vector.tensor_copy(carry[:, s:s + 1],
                                                  h[:, SEG - 1:SEG])
                        g = plg.tile([DI, SEG], BF16, tag="g")
                        nc.vector.tensor_mul(g[:], h[:], cbc[:])
                        for j, cix in enumerate(SEG_CHUNKS[q]):
                            nc.tensor.matmul(
                                y_ps[cix][:DI], ident_g[:],
                                g[:, j * CH:(j + 1) * CH],
                                start=False, stop=(s == DS - 1))
                        if s + PREF < DS:
                            bc_issue(q, s + PREF)
                        if s in work:
                            work[s]()

                # =========== emission ===========
                front_chunk(0, False)
                # dummy 8-core sync: absorbs core launch skew while the
                # front runs, so tail collectives don't pay it
                nc.gpsimd.memset(stats_a[:], 0.0)
                nc.sync.dma_start(sync_in[:], stats_a[:])
                nc.gpsimd.collective_compute(
                    "AllReduce", OP.add, replica_groups=G8,
                    ins=[sync_in[:].opt()], outs=[sync_out[:].opt()])
                front_chunk(1, False)
                finish_front((0, 1))

                seg_prefetch(0)
                seg_scan(0, {2: lambda: front_chunk(2, True),
                             6: lambda: front_chunk(3, True),
                             10: lambda: seg_prefetch(1)})
                seg_scan(1, {2: lambda: front_chunk(4, True),
                             5: lambda: front_chunk(5, True),
                             8: lambda: front_chunk(6, True),
                             11: lambda: front_chunk(7, True),
                             13: lambda: seg_prefetch(2)})
                wave(0)
                seg_scan(2, {8: lambda: conv3_chunk(0),
                             11: lambda: conv3_chunk(1),
                             13: lambda: seg_prefetch(3),
                             14: lambda: conv3_chunk(2)})
                wave(1)
                seg_scan(3, {})
                wave(2)
                # conv 3-5 hide the last exchange's latency on the PE queue
                conv3_chunk(3)
                conv3_chunk(4)
                conv3_chunk(5)
                stats_early()
                conv3_chunk(6)
                conv3_chunk(7)

                # ---- batch stats AllReduces + BN + residual + leaky ----
                tl = smid
                stot = tl.tile([C, 2], F32)
                stot_b = tl.tile([C, 2], F32)
                stats = tl.tile([C, 2], F32)
                nc.vector.tensor_reduce(stats[:, 0:1], stats_m[:, 6:8],
                                        axis=mybir.AxisListType.X, op=OP.add)
                nc.vector.tensor_reduce(stats[:, 1:2], stats_v[:, 6:8],
                                        axis=mybir.AxisListType.X, op=OP.add)
                nc.sync.dma_start(st_in_b[:], stats[:])
                nc.gpsimd.collective_compute(
                    "AllReduce", OP.add, replica_groups=G8,
                    ins=[st_in_b[:].opt()], outs=[st_out_b[:].opt()])
                nc.sync.dma_start(stot[:], st_out_a[:])
                nc.sync.dma_start(stot_b[:], st_out_b[:])
                nc.vector.tensor_add(stot[:], stot[:], stot_b[:])

                # every sample's full conv is present on both pair cores,
                # so the 8-core sum double counts: divide by 2*B*L
                inv = 1.0 / (2.0 * B * L)
                mean = tl.tile([C, 1], F32)
                ex2 = tl.tile([C, 1], F32)
                var = tl.tile([C, 1], F32)
                tmp = tl.tile([C, 1], F32)
                nc.vector.tensor_scalar_mul(mean[:], stot[:, 0:1], inv)
                nc.vector.tensor_scalar_mul(ex2[:], stot[:, 1:2], inv)
                nc.vector.tensor_mul(tmp[:], mean[:], mean[:])
                nc.vector.tensor_sub(var[:], ex2[:], tmp[:])
                # invstd = exp(-0.5*ln(var+eps)) -- ln/exp stay in the
                # loaded table set (no sqrt-set reload on the tail)
                nc.vector.tensor_scalar_add(var[:], var[:], 1e-5)
                nc.scalar.activation(tmp[:], var[:], AF.Ln)
                nc.scalar.activation(tmp[:], tmp[:], AF.Exp, scale=-0.5)
                scal = tl.tile([C, 1], F32)
                shft = tl.tile([C, 1], F32)
                nc.vector.tensor_mul(scal[:], p_bng[:], tmp[:])
                nc.vector.tensor_mul(tmp[:], mean[:], scal[:])
                nc.vector.tensor_sub(shft[:], p_bnb[:], tmp[:])

                # bn + residual + leaky relu: out = prelu(conv*scal + res
                # + shft); conv*scal on ACT (per-partition scale), add on
                # DVE at 2x, prelu+shift on ACT straight to f32 out
                for lo in range(0, L, 1024):
                    hi = lo + 1024
                    bs = plf.tile([C, 1024], BF16, tag="bn")
                    nc.scalar.activation(bs[:], conv_sb[:, lo:hi],
                                         AF.Copy, scale=scal[:, 0:1])
                    nc.vector.tensor_add(bs[:], bs[:], res_sb[:, lo:hi])
                    ot = plf.tile([C, 1024], F32, tag="ot")
                    nc.scalar.activation(ot[:], bs[:],
                                         AF.Prelu, alpha=0.01,
                                         bias=shft[:, 0:1])
                    nc.sync.dma_start(out_d[:, lo:hi], ot[:])

    nc.compile()
    return nc


_NC = None


def _get_nc():
    global _NC
    if _NC is None:
        _NC = _build()
    return _NC


def _prep_in_maps(inp):
    inp = {k: np.asarray(v, dtype=np.float32) for k, v in inp.items()}
    x = inp["x"]  # (4, 64, 64, 64)
    # full 3x3 conv weights over both direction blocks, [in=128, 9*64]
    c3 = np.zeros((128, 9 * C), np.float32)
    for ky in range(3):
        for kx in range(3):
            c3[:, (ky * 3 + kx) * C:(ky * 3 + kx + 1) * C] = \
                inp["conv_w"][:, :, ky, kx].T
    maps = []
    for core in range(NCORE):
        b, d = core // 2, core % 2
        pre = "m1_" if d == 0 else "m2_"
        in_w = inp[pre + "in_w"]          # (256, 64)
        xproj_w = inp[pre + "xproj_w"]    # (36, 128)
        dt_w = inp[pre + "dt_w"]          # (128, 4)
        conv1_w = inp[pre + "conv_w"]     # (128, 4)

        x_loc = x[b].reshape(C, L)
        if d == 1:
            x_loc = x_loc[:, ::-1]

        bigproj = dt_w @ xproj_w[:DTR]    # (128, 128)

        blob_f = np.zeros((128, BF_COLS), np.float32)
        # fused in-projection + depthwise causal conv:
        # W_k[ch_x, di] = in_w[di, ch_x] * conv1_w[di, k]
        xi_w = in_w[:DI]                  # (128, 64)
        for k in range(DCONV):
            blob_f[:C, 128 * k:128 * (k + 1)] = \
                (xi_w * conv1_w[:, k:k + 1]).T
        blob_f[:C, 512:640] = in_w[DI:].T
        blob_f[:C, 640:704] = inp["res_w"].T
        blob_f[:, 704] = inp[pre + "conv_b"]
        blob_f[:, 705] = inp[pre + "dt_b"]
        blob_f[:, 706:722] = -np.exp(inp[pre + "A_log"])
        blob_f[:, 722] = inp[pre + "D"]
        blob_f[:C, 723] = inp["conv_b"]
        blob_f[:C, 724] = inp["res_b"]
        blob_f[:C, 725] = inp["bn_gamma"]
        blob_f[:C, 726] = inp["bn_beta"]
        blob_f[:, 728] = -inp[pre + "conv_b"]
        blob_h = np.zeros((128, BH_COLS), np.float32)
        blob_h[:, 0:9 * C] = c3
        blob_h[:, 9 * C:9 * C + C] = inp[pre + "out_w"].T
        blob_h[:, 9 * C + C:9 * C + C + 128] = bigproj.T
        blob_h[:, 9 * C + C + 128:9 * C + C + 160] = xproj_w[DTR:].T
        m = {
            "x_loc": np.ascontiguousarray(x_loc),
            "blob_f": blob_f,
            "blob_h": blob_h.astype(ml_dtypes.bfloat16),
        }
        maps.append(m)
    return maps


def _run(inputs, trace=False):
    nc = _get_nc()
    maps = _prep_in_maps(inputs)
    res = bass_utils.run_bass_kernel_spmd(
        nc, maps, core_ids=list(range(NCORE)), trace=trace)
    out = np.stack([res.results[2 * b]["out"].reshape(C, H, W)
                    for b in range(B)])
    return out, res


def kernel(**inputs) -> np.ndarray:
    out, _ = _run(inputs, trace=False)
    return out


# revision 36
# speedup vs baseline: 1.1823x; 1.1823x over previous
"""BiMamba block kernel for 8 Trainium2 NeuronCores.

Sharding: core = 2*sample + direction (4 samples x 2 scan directions).
Each core runs the full mamba for its (sample, direction).

The selective scan dominates on DVE (16 states x 4096 cols at ~2.1
ns/col is irreducible), so the kernel keeps the DVE queue dense and
hides everything else under it:

- All activations use only the natural_log_exp ACT table set: silu is
  computed as v*sigmoid(v) with sigmoid(v) = exp(-softplus(-v))
  (3 exp/ln ops + one DVE scalar_tensor_tensor), so no table reload
  ever interrupts the exp stream of the scan.
- Scan: 4 segments (1024/1024/1536/512 cols); the first scan starts as
  soon as front chunks 0-1 are projected (~20 us).  Remaining front
  chunks are interleaved between scan states.  B/C rows arrive via
  DRAM partition-broadcast DMAs issued 5 states ahead.
- Exchange: after segments 1/2/3 the out-projection rows are
  AllGathered with the pair core (rank order = [dir0; dir1], matching
  the reference's un-unflipped y2 concat), giving every core the full
  128-channel conv input; the 3x3 conv runs locally, no collective
  after it.  A dummy 8-core AllReduce early in the kernel absorbs the
  core launch skew so the tail stats AllReduce doesn't pay it.
- BatchNorm stats are split: chunks 0-5 AllReduce overlaps the last
  conv chunks; only the tiny chunk 6-7 reduce sits on the tail.
  invstd via ln/exp (no sqrt table load).
"""
import os
import sys

for _p in ("/opt/trn_rl_repo", "/root/.axon_site/_ro/trn_rl_repo"):
    if os.path.isdir(_p):
        if _p not in sys.path:
            sys.path.insert(0, _p)
        break

import ml_dtypes
import numpy as np

# The agent image's antenv lacks axon_hooks; inject it so trace=True can
# capture NTFF profiles (used by test.py for HW timing, not for grading).
try:
    import antenv.axon_hooks  # noqa: F401
except ImportError:
    try:
        import types as _types

        from trn_agent_boot.trn_boot import _ntff_profile_via_ctypes

        _hook = _ntff_profile_via_ctypes("/opt/axon/libaxon_pjrt.so")
        _m = _types.ModuleType("antenv.axon_hooks")
        _m.get_axon_ntff_profile_hook = lambda: _hook
        _m.set_axon_ntff_profile_hook = lambda h: None
        sys.modules["antenv.axon_hooks"] = _m
    except Exception:
        pass

import concourse.bass as bass
import concourse.mybir as mybir
from concourse import bacc
from concourse import bass_utils
from concourse.masks import make_identity
from concourse.tile import TileContext

F32 = mybir.dt.float32
BF16 = mybir.dt.bfloat16
AF = mybir.ActivationFunctionType
OP = mybir.AluOpType

B, C, H, W = 4, 64, 64, 64
L = H * W          # 4096
DI = 128           # d_inner
DS = 16            # d_state
DTR = 4            # dt_rank
DCONV = 4
NCORE = 8
CH = 512           # matmul free-dim chunk
NCH = L // CH      # 8
RPC = CH // W      # output rows per chunk (8)

SEGS = ((0, 2048), (2048, 3584), (3584, 4096))
SEG_CHUNKS = ((0, 1, 2, 3), (4, 5, 6), (7,))
NSEG = len(SEGS)
# exchange waves: one per segment, covering its chunks
WAVES = ((0, (0, 1, 2, 3)), (1, (4, 5, 6)), (2, (7,)))
# conv pieces (row ranges): pieces 0-6 need only waves 0-1; pieces 7-8
# (rows 55-63, PSUM-bank-sized) are the only ones gated on the last
# exchange
CONV_PIECES = ((0, 8), (8, 16), (16, 24), (24, 32), (32, 40), (40, 48),
               (48, 55), (55, 63), (63, 64))
NPIECE = len(CONV_PIECES)

BH_COLS = 9 * C + C + 128 + 32   # c3w | owT | bigT | bcwT
BF_COLS = 736


def _build():
    nc = bacc.Bacc(target_bir_lowering=False, debug=False, num_devices=NCORE)

    def din(name, shape, dtype=F32):
        return nc.dram_tensor(name, shape, dtype, kind="ExternalInput")

    F32R = mybir.dt.float32r
    x_loc = din("x_loc", [C, L], F32R)
    blob_f = din("blob_f", [128, BF_COLS], F32R)
    blob_h = din("blob_h", [128, BH_COLS], BF16)

    out_d = nc.dram_tensor("out", [C, L], F32, kind="ExternalOutput")

    with TileContext(nc) as tc:
        with tc.tile_pool(name="pers", bufs=1) as pers:
            # ---- params arrive as two packed blobs ----
            p_bf = pers.tile([128, BF_COLS], F32R)
            p_bh = pers.tile([128, BH_COLS], BF16)
            nc.sync.dma_start(p_bf[:], blob_f[:])
            nc.sync.dma_start(p_bh[:], blob_h[:])
            p_wk = [p_bf[:, 128 * k:128 * (k + 1)] for k in range(DCONV)]
            p_zwT = p_bf[:, 512:640]
            p_rwT = p_bf[:, 640:704]
            p_c1b = p_bf[:, 704:705].bitcast(F32)
            p_dtb = p_bf[:, 705:706].bitcast(F32)
            p_A = p_bf[:, 706:722].bitcast(F32)
            p_D = p_bf[:, 722:723].bitcast(F32)
            p_c3b = p_bf[:C, 723:724].bitcast(F32)
            p_rb = p_bf[:C, 724:725].bitcast(F32)
            p_bng = p_bf[:C, 725:726].bitcast(F32)
            p_bnb = p_bf[:C, 726:727].bitcast(F32)
            p_nc1b = p_bf[:, 728:729].bitcast(F32)   # -conv1 bias
            p_c3w = p_bh[:, 0:9 * C]
            p_owT = p_bh[:, 9 * C:9 * C + C]
            p_bigT = p_bh[:, 9 * C + C:9 * C + C + 128]
            p_bcwT = p_bh[:, 9 * C + C + 128:9 * C + C + 160]

            ident = pers.tile([128, 128], F32)
            make_identity(nc, ident[:])
            ident_g = pers.tile([128, 128], BF16)
            nc.vector.tensor_copy(ident_g[:], ident[:])

            # DRAM staging for B/C rows (DMA partition-broadcast needs a
            # DRAM source)
            bc_dram = nc.dram_tensor("bc_stage", [2 * DS, L], BF16)

            x_pad = pers.tile([64, 3 + L], F32R)
            nc.gpsimd.memset(x_pad[:, 0:3].bitcast(F32), 0.0)
            # split load so the front chunks 0-3 start without waiting for
            # the full x (chunk 3's last tap reads through x col 2048)
            nc.sync.dma_start(x_pad[:, 3:3 + 1040], x_loc[:, 0:1040])
            nc.sync.dma_start(x_pad[:, 3 + 1040:3 + 2064],
                              x_loc[:, 1040:2064])
            nc.sync.dma_start(x_pad[:, 3 + 2064:3 + L], x_loc[:, 2064:L])

            with tc.tile_pool(name="smid", bufs=1) as smid, \
                 tc.tile_pool(name="ps", bufs=4, space="PSUM") as psp, \
                 tc.tile_pool(name="psy", bufs=4, space="PSUM") as psy, \
                 tc.tile_pool(name="sl_e", bufs=2) as plex, \
                 tc.tile_pool(name="sl_a", bufs=3) as pla, \
                 tc.tile_pool(name="sl_b", bufs=5) as plb, \
                 tc.tile_pool(name="sl_x", bufs=3) as plx, \
                 tc.tile_pool(name="sl_h", bufs=3) as plh, \
                 tc.tile_pool(name="sl_c", bufs=5) as plc, \
                 tc.tile_pool(name="sl_g", bufs=3) as plg, \
                 tc.tile_pool(name="sl_f", bufs=2) as plf, \
                 tc.tile_pool(name="dram", bufs=1, space="DRAM") as dr:
                z_sil = smid.tile([DI, L], BF16)
                dtv = smid.tile([DI, L], BF16)
                dtxc = smid.tile([DI, L], BF16)
                xcd = smid.tile([DI, L], BF16)
                xc = smid.tile([DI, L], BF16)
                carry = smid.tile([DI, DS], F32)

                ympad = smid.tile([128, H + 2, W + 2], BF16)
                nc.gpsimd.memset(ympad[:], 0.0)
                res_sb = smid.tile([C, L], BF16)
                conv_sb = smid.tile([C, L], BF16)
                stats_m = smid.tile([C, NPIECE], F32)
                stats_v = smid.tile([C, NPIECE], F32)
                PAIRS = [[0, 1], [2, 3], [4, 5], [6, 7]]
                G8 = [[0, 1, 2, 3, 4, 5, 6, 7]]

                cc_ins, cc_outs = [], []
                for wi, (_, cvs) in enumerate(WAVES):
                    cc_ins.append(dr.tile([C, len(cvs) * CH], BF16,
                                          name=f"cci{wi}"))
                    cc_outs.append(dr.tile([128, len(cvs) * CH], BF16,
                                           name=f"cco{wi}"))
                st_in_a = dr.tile([C, 2], F32, name="st_in_a")
                st_in_b = dr.tile([C, 2], F32, name="st_in_b")
                sync_in = dr.tile([C, 2], F32, name="sync_in")
                st_out_a = nc.dram_tensor("st_out_a", [C, 2], F32,
                                          addr_space="Shared")
                st_out_b = nc.dram_tensor("st_out_b", [C, 2], F32,
                                          addr_space="Shared")
                sync_out = nc.dram_tensor("sync_out", [C, 2], F32,
                                          addr_space="Shared")

                def sigmoid_mul(dst_sl, ps_t, bias, nbias):
                    """dst = (v+b)*sigmoid(v+b) with sigmoid computed as
                    exp(-ln(1+exp(-(v+b)))) -- exp/ln only, one table set."""
                    t = plf.tile([DI, CH], BF16, tag="sg")
                    if nbias is not None:
                        nc.scalar.activation(t[:], ps_t[:DI], AF.Exp,
                                             scale=-1.0, bias=nbias[:, 0:1])
                    else:
                        nc.scalar.activation(t[:], ps_t[:DI], AF.Exp,
                                             scale=-1.0)
                    nc.scalar.activation(t[:], t[:], AF.Ln, bias=1.0)
                    nc.scalar.activation(t[:], t[:], AF.Exp, scale=-1.0)
                    nc.vector.scalar_tensor_tensor(
                        dst_sl, ps_t[:DI],
                        bias[:, 0:1] if bias is not None else 0.0,
                        t[:], op0=OP.add, op1=OP.mult)

                def front_chunk(c, with_ln):
                    """Full front for chunk c: fused in-proj + causal conv,
                    gate, silu, dt/B/C projections."""
                    sl = slice(c * CH, (c + 1) * CH)
                    ps = psp.tile([128, CH], F32, tag="ps")
                    for k in range(DCONV):
                        nc.tensor.matmul(ps[:DI], p_wk[k][:C],
                                         x_pad[:, c * CH + k:c * CH + k + CH],
                                         start=(k == 0), stop=(k == DCONV - 1))
                    ps2 = psp.tile([128, CH], F32, tag="ps")
                    nc.tensor.matmul(ps2[:DI], p_zwT[:C],
                                     x_pad[:, 3 + c * CH:3 + (c + 1) * CH],
                                     start=True, stop=True)
                    sigmoid_mul(xc[:, sl], ps, p_c1b, p_nc1b)
                    sigmoid_mul(z_sil[:, sl], ps2, None, None)
                    ps3 = psp.tile([128, CH], F32, tag="ps")
                    nc.tensor.matmul(ps3[:DI], p_bigT[:], xc[:, sl],
                                     start=True, stop=True)
                    nc.scalar.activation(dtv[:, sl], ps3[:DI], AF.Exp,
                                         bias=p_dtb[:, 0:1])
                    ps4 = psp.tile([128, CH], F32, tag="ps")
                    nc.tensor.matmul(ps4[:2 * DS], p_bcwT[:], xc[:, sl],
                                     start=True, stop=True)
                    bch = plb.tile([2 * DS, CH], BF16, tag="bch")
                    nc.scalar.copy(bch[:], ps4[:2 * DS])
                    nc.sync.dma_start(bc_dram[:, sl], bch[:])
                    if with_ln:
                        nc.scalar.activation(dtv[:, sl], dtv[:, sl], AF.Ln,
                                             bias=1.0)
                        nc.vector.tensor_mul(dtxc[:, sl], dtv[:, sl],
                                             xc[:, sl])
                        nc.scalar.activation(xcd[:, sl], xc[:, sl],
                                             AF.Copy, scale=p_D[:, 0:1])

                def finish_front(cs):
                    hsl = slice(cs[0] * CH, (cs[-1] + 1) * CH)
                    nc.scalar.activation(dtv[:, hsl], dtv[:, hsl], AF.Ln,
                                         bias=1.0)
                    for c in cs:
                        sl = slice(c * CH, (c + 1) * CH)
                        nc.vector.tensor_mul(dtxc[:, sl], dtv[:, sl],
                                             xc[:, sl])
                        nc.scalar.activation(xcd[:, sl], xc[:, sl],
                                             AF.Copy, scale=p_D[:, 0:1])

                def wave(wi):
                    """Out-projection + pair AllGather + ympad write +
                    residual for the wave's chunks.  AllGather output is
                    rank-ordered, so both cores get [dir0; dir1]."""
                    cvs = WAVES[wi][1]
                    stage = plex.tile([C, len(cvs) * CH], BF16,
                                      tag="stage", name=f"stage{wi}")
                    for j, cix in enumerate(cvs):
                        sl = slice(cix * CH, (cix + 1) * CH)
                        ssl = slice(j * CH, (j + 1) * CH)
                        yg = plf.tile([DI, CH], BF16, tag="yg")
                        nc.vector.tensor_mul(yg[:], y_ps[cix][:DI],
                                             z_sil[:, sl])
                        po = psp.tile([128, CH], F32, tag="ps",
                                      name=f"po{cix}")
                        nc.tensor.matmul(po[:C], p_owT[:], yg[:],
                                         start=True, stop=True)
                        nc.scalar.copy(stage[:, ssl], po[:C])
                        psr = psp.tile([128, CH], F32, tag="ps",
                                       name=f"rs{cix}")
                        nc.tensor.matmul(psr[:C], p_rwT[:C],
                                         x_pad[:, 3 + cix * CH:
                                               3 + (cix + 1) * CH],
                                         start=True, stop=True)
                        nc.scalar.activation(res_sb[:, sl], psr[:C],
                                             AF.Identity, bias=p_rb[:, 0:1])
                    nc.sync.dma_start(cc_ins[wi][:], stage[:])
                    nc.gpsimd.collective_compute(
                        "AllGather", OP.bypass, replica_groups=PAIRS,
                        ins=[cc_ins[wi][:].opt()], outs=[cc_outs[wi][:].opt()])
                    r0 = cvs[0] * RPC
                    nrows = len(cvs) * RPC
                    nc.sync.dma_start(
                        ympad[:, 1 + r0:1 + r0 + nrows, 1:1 + W],
                        cc_outs[wi][:].rearrange("p (r w) -> p r w", w=W))

                def conv3_piece(i):
                    r0, r1 = CONV_PIECES[i]
                    nr = r1 - r0
                    ps = psp.tile([128, nr * W], F32, tag="ps",
                                  name=f"cv{i}")
                    ps3 = ps[:C].rearrange("p (r w) -> p r w", w=W)
                    n = 0
                    for ky in range(3):
                        for kx in range(3):
                            nc.tensor.matmul(
                                ps3[:],
                                p_c3w[:, (ky * 3 + kx) * C:
                                      (ky * 3 + kx + 1) * C],
                                ympad[:, r0 + ky:r0 + ky + nr, kx:kx + W],
                                start=(n == 0), stop=(n == 8))
                            n += 1
                    sl = slice(r0 * W, r1 * W)
                    flat = ps3.rearrange("p r w -> p (r w)")
                    nc.scalar.activation(conv_sb[:, sl], flat,
                                         AF.Identity, bias=p_c3b[:, 0:1],
                                         accum_out=stats_m[:, i:i + 1])
                    sq = plf.tile([C, nr * W], BF16, tag="sq")
                    nc.scalar.activation(sq[:], conv_sb[:, sl],
                                         AF.Square,
                                         accum_out=stats_v[:, i:i + 1])

                stats_a = smid.tile([C, 2], F32)

                def stats_early():
                    nc.vector.tensor_reduce(stats_a[:, 0:1],
                                            stats_m[:, 0:7],
                                            axis=mybir.AxisListType.X,
                                            op=OP.add)
                    nc.vector.tensor_reduce(stats_a[:, 1:2],
                                            stats_v[:, 0:7],
                                            axis=mybir.AxisListType.X,
                                            op=OP.add)
                    nc.sync.dma_start(st_in_a[:], stats_a[:])
                    nc.gpsimd.collective_compute(
                        "AllReduce", OP.add, replica_groups=G8,
                        ins=[st_in_a[:].opt()], outs=[st_out_a[:].opt()])

                y_ps = {}
                bc_q = {}
                PREF = 5

                def bc_issue(q, s):
                    t0, t1 = SEGS[q]
                    SEG = t1 - t0
                    qsl = slice(t0, t1)
                    bbc = plb.tile([DI, SEG], BF16, tag="bbc",
                                   name=f"bbc{q}_{s}")
                    nc.sync.dma_start(
                        bbc[:],
                        bc_dram[s:s + 1, qsl].to_broadcast((DI, SEG)))
                    cbc = plc.tile([DI, SEG], BF16, tag="cbc",
                                   name=f"cbc{q}_{s}")
                    nc.sync.dma_start(
                        cbc[:],
                        bc_dram[DS + s:DS + s + 1, qsl].to_broadcast(
                            (DI, SEG)))
                    bc_q[(q, s)] = (bbc, cbc)

                def seg_prefetch(q):
                    for s in range(PREF):
                        bc_issue(q, s)

                def seg_scan(q, work):
                    t0, t1 = SEGS[q]
                    SEG = t1 - t0
                    qsl = slice(t0, t1)
                    for cix in SEG_CHUNKS[q]:
                        yp = psy.tile([128, CH], F32, tag="yps",
                                      name=f"y{cix}")
                        nc.tensor.matmul(yp[:DI], ident_g[:],
                                         xcd[:, cix * CH:(cix + 1) * CH],
                                         start=True, stop=False)
                        y_ps[cix] = yp
                    for s in range(DS):
                        da = pla.tile([DI, SEG], BF16, tag="da")
                        nc.scalar.activation(da[:], dtv[:, qsl], AF.Exp,
                                             scale=p_A[:, s:s + 1])
                        bbc, cbc = bc_q.pop((q, s))
                        dbx = plx.tile([DI, SEG], BF16, tag="dbx")
                        nc.vector.tensor_mul(dbx[:], dtxc[:, qsl], bbc[:])
                        h = plh.tile([DI, SEG], BF16, tag="h")
                        init = 0.0 if q == 0 else carry[:, s:s + 1]
                        nc.vector.tensor_tensor_scan(h[:], da[:], dbx[:],
                                                     init, op0=OP.mult,
                                                     op1=OP.add)
                        if q < NSEG - 1:
                            # on DVE so the in-order ACT queue of exps is
                            # never blocked behind a scan result
                            nc.vector.tensor_copy(carry[:, s:s + 1],
                                                  h[:, SEG - 1:SEG])
                        g = plg.tile([DI, SEG], BF16, tag="g")
                        nc.vector.tensor_mul(g[:], h[:], cbc[:])
                        for j, cix in enumerate(SEG_CHUNKS[q]):
                            nc.tensor.matmul(
                                y_ps[cix][:DI], ident_g[:],
                                g[:, j * CH:(j + 1) * CH],
                                start=False, stop=(s == DS - 1))
                        if s + PREF < DS:
                            bc_issue(q, s + PREF)
                        if s in work:
                            work[s]()

                # =========== emission ===========
                front_chunk(0, False)
                # dummy 8-core sync: absorbs core launch skew while the
                # front runs, so tail collectives don't pay it
                nc.gpsimd.memset(stats_a[:], 0.0)
                nc.sync.dma_start(sync_in[:], stats_a[:])
                nc.gpsimd.collective_compute(
                    "AllReduce", OP.add, replica_groups=G8,
                    ins=[sync_in[:].opt()], outs=[sync_out[:].opt()])
                front_chunk(1, False)
                front_chunk(2, False)
                front_chunk(3, False)
                finish_front((0, 1, 2, 3))

                seg_prefetch(0)
                seg_scan(0, {2: lambda: front_chunk(4, True),
                             5: lambda: front_chunk(5, True),
                             8: lambda: front_chunk(6, True),
                             11: lambda: front_chunk(7, True),
                             13: lambda: seg_prefetch(1)})
                wave(0)
                seg_scan(1, {6: lambda: conv3_piece(0),
                             9: lambda: conv3_piece(1),
                             12: lambda: conv3_piece(2),
                             13: lambda: seg_prefetch(2)})
                wave(1)
                seg_scan(2, {10: lambda: conv3_piece(3),
                             12: lambda: conv3_piece(4),
                             14: lambda: conv3_piece(5)})
                wave(2)
                # piece 6 hides the last exchange's latency on the PE queue
                conv3_piece(6)
                stats_early()
                conv3_piece(7)
                conv3_piece(8)

                # ---- batch stats AllReduces + BN + residual + leaky ----
                tl = smid
                stot = tl.tile([C, 2], F32)
                stot_b = tl.tile([C, 2], F32)
                stats = tl.tile([C, 2], F32)
                nc.vector.tensor_reduce(stats[:, 0:1], stats_m[:, 7:9],
                                        axis=mybir.AxisListType.X, op=OP.add)
                nc.vector.tensor_reduce(stats[:, 1:2], stats_v[:, 7:9],
                                        axis=mybir.AxisListType.X, op=OP.add)
                nc.sync.dma_start(st_in_b[:], stats[:])
                nc.gpsimd.collective_compute(
                    "AllReduce", OP.add, replica_groups=G8,
                    ins=[st_in_b[:].opt()], outs=[st_out_b[:].opt()])
                nc.sync.dma_start(stot[:], st_out_a[:])
                nc.sync.dma_start(stot_b[:], st_out_b[:])
                nc.vector.tensor_add(stot[:], stot[:], stot_b[:])

                # every sample's full conv is present on both pair cores,
                # so the 8-core sum double counts: divide by 2*B*L
                inv = 1.0 / (2.0 * B * L)
                mean = tl.tile([C, 1], F32)
                ex2 = tl.tile([C, 1], F32)
                var = tl.tile([C, 1], F32)
                tmp = tl.tile([C, 1], F32)
                nc.vector.tensor_scalar_mul(mean[:], stot[:, 0:1], inv)
                nc.vector.tensor_scalar_mul(ex2[:], stot[:, 1:2], inv)
                nc.vector.tensor_mul(tmp[:], mean[:], mean[:])
                nc.vector.tensor_sub(var[:], ex2[:], tmp[:])
                # invstd = exp(-0.5*ln(var+eps)) -- ln/exp stay in the
                # loaded table set (no sqrt-set reload on the tail)
                nc.vector.tensor_scalar_add(var[:], var[:], 1e-5)
                nc.scalar.activation(tmp[:], var[:], AF.Ln)
                nc.scalar.activation(tmp[:], tmp[:], AF.Exp, scale=-0.5)
                scal = tl.tile([C, 1], F32)
                shft = tl.tile([C, 1], F32)
                nc.vector.tensor_mul(scal[:], p_bng[:], tmp[:])
                nc.vector.tensor_mul(tmp[:], mean[:], scal[:])
                nc.vector.tensor_sub(shft[:], p_bnb[:], tmp[:])

                # bn + residual + leaky relu: out = prelu(conv*scal + res
                # + shft); conv*scal on ACT (per-partition scale), add on
                # DVE at 2x, prelu+shift on ACT straight to f32 out
                for lo in range(0, L, 1024):
                    hi = lo + 1024
                    bs = plf.tile([C, 1024], BF16, tag="bn")
                    nc.scalar.activation(bs[:], conv_sb[:, lo:hi],
                                         AF.Copy, scale=scal[:, 0:1])
                    nc.vector.tensor_add(bs[:], bs[:], res_sb[:, lo:hi])
                    ot = plf.tile([C, 1024], F32, tag="ot")
                    nc.scalar.activation(ot[:], bs[:],
                                         AF.Prelu, alpha=0.01,
                                         bias=shft[:, 0:1])
                    nc.sync.dma_start(out_d[:, lo:hi], ot[:])

    nc.compile()
    return nc


_NC = None


def _get_nc():
    global _NC
    if _NC is None:
        _NC = _build()
    return _NC


def _prep_in_maps(inp):
    inp = {k: np.asarray(v, dtype=np.float32) for k, v in inp.items()}
    x = inp["x"]  # (4, 64, 64, 64)
    # full 3x3 conv weights over both direction blocks, [in=128, 9*64]
    c3 = np.zeros((128, 9 * C), np.float32)
    for ky in range(3):
        for kx in range(3):
            c3[:, (ky * 3 + kx) * C:(ky * 3 + kx + 1) * C] = \
                inp["conv_w"][:, :, ky, kx].T
    maps = []
    for core in range(NCORE):
        b, d = core // 2, core % 2
        pre = "m1_" if d == 0 else "m2_"
        in_w = inp[pre + "in_w"]          # (256, 64)
        xproj_w = inp[pre + "xproj_w"]    # (36, 128)
        dt_w = inp[pre + "dt_w"]          # (128, 4)
        conv1_w = inp[pre + "conv_w"]     # (128, 4)

        x_loc = x[b].reshape(C, L)
        if d == 1:
            x_loc = x_loc[:, ::-1]

        bigproj = dt_w @ xproj_w[:DTR]    # (128, 128)

        blob_f = np.zeros((128, BF_COLS), np.float32)
        # fused in-projection + depthwise causal conv:
        # W_k[ch_x, di] = in_w[di, ch_x] * conv1_w[di, k]
        xi_w = in_w[:DI]                  # (128, 64)
        for k in range(DCONV):
            blob_f[:C, 128 * k:128 * (k + 1)] = \
                (xi_w * conv1_w[:, k:k + 1]).T
        blob_f[:C, 512:640] = in_w[DI:].T
        blob_f[:C, 640:704] = inp["res_w"].T
        blob_f[:, 704] = inp[pre + "conv_b"]
        blob_f[:, 705] = inp[pre + "dt_b"]
        blob_f[:, 706:722] = -np.exp(inp[pre + "A_log"])
        blob_f[:, 722] = inp[pre + "D"]
        blob_f[:C, 723] = inp["conv_b"]
        blob_f[:C, 724] = inp["res_b"]
        blob_f[:C, 725] = inp["bn_gamma"]
        blob_f[:C, 726] = inp["bn_beta"]
        blob_f[:, 728] = -inp[pre + "conv_b"]
        blob_h = np.zeros((128, BH_COLS), np.float32)
        blob_h[:, 0:9 * C] = c3
        blob_h[:, 9 * C:9 * C + C] = inp[pre + "out_w"].T
        blob_h[:, 9 * C + C:9 * C + C + 128] = bigproj.T
        blob_h[:, 9 * C + C + 128:9 * C + C + 160] = xproj_w[DTR:].T
        m = {
            "x_loc": np.ascontiguousarray(x_loc),
            "blob_f": blob_f,
            "blob_h": blob_h.astype(ml_dtypes.bfloat16),
        }
        maps.append(m)
    return maps


def _run(inputs, trace=False):
    nc = _get_nc()
    maps = _prep_in_maps(inputs)
    res = bass_utils.run_bass_kernel_spmd(
        nc, maps, core_ids=list(range(NCORE)), trace=trace)
    out = np.stack([res.results[2 * b]["out"].reshape(C, H, W)
                    for b in range(B)])
    return out, res


def kernel(**inputs) -> np.ndarray:
    out, _ = _run(inputs, trace=False)
    return out
